# revision 1
# baseline (speedup 1.0000x reference)
"""Trainium2 Bass kernel for nn_BertAdapterCapsuleMask.

Strategy (8 NeuronCores, SPMD — identical program, per-core data):

The reference computes, per example b:
  sem   = squash_n(x @ sem_w + sem_b)                      (capsule layer)
  priors[c,n,:] = sem[n,:] @ route_weights[c,n]            (routing priors)
  vote  = 3-iter masked dynamic routing over (c,b) pairs
  h_out = reshape(vote,(B,S,C)) @ larger_w' + larger_b'    (NB: the reshape
          mixes examples: h_out[b] reads vote rows 3b..3b+2 of the
          row-major [C*B, S] vote matrix)
  out   = x + adapter(x + h_out)                           (768->2048->768 MLP)

Sharding: the routing problem is independent per (c,b) pair (384 pairs).
Core k owns pairs t in [48k, 48k+48) AND examples b in [16k, 16k+16).
Because vote row index used by h_out[b] is exactly 3b..3b+2, core k's own
pairs produce precisely the vote rows its own examples need — zero
cross-core communication.  Each core computes sem for the 48 examples
b' = t mod 128 its pairs reference (sem is cheap), then routing, then the
adapter for its 16 own examples.

Weight folds (host side, exact):
  gfc1 folded into fc2_w rows;  glarger into larger_w;  (larger_b*glarger)
  and larger_b's path folded into fc1_b;  h_out@fc1_w folded to
  V @ M1 with M1 = (larger_w*glarger) @ fc1_w, so h_out never materializes.

Precision: matmuls in bf16 (fp32 PSUM accumulation), routing arithmetic in
fp32, final residual adds the untouched fp32 x.
"""

import numpy as np
import ml_dtypes

import concourse.bass as bass
import concourse.bacc as bacc
import concourse.mybir as mybir
import concourse.tile as tile
from concourse import bass_utils

BF16 = ml_dtypes.bfloat16
F32 = mybir.dt.float32
BF = mybir.dt.bfloat16
AF = mybir.ActivationFunctionType
ALU = mybir.AluOpType

B, S, H, A, C, N = 128, 128, 768, 2048, 3, 10
NUM_ITER = 3
NCORES = 8
BL = B // NCORES          # 16 own examples / core
NPAIR = 3 * B // NCORES   # 48 routing pairs / core
HK = H // 128             # 6
AK = A // 128             # 16
TOK = BL * S              # 2048 tokens / core
HALF = TOK // 2           # 1024
NC30 = N * C              # 30
NSEM = 18                 # sem examples per core: [16k, 16k+18) mod 128


def _sigmoid_f32(z):
    z = np.asarray(z, np.float32)
    out = np.empty_like(z)
    pos = z >= 0
    out[pos] = 1.0 / (1.0 + np.exp(-z[pos], dtype=np.float32))
    ez = np.exp(z[~pos], dtype=np.float32)
    out[~pos] = ez / (1.0 + ez)
    return out.astype(np.float32)


def _bf(x):
    return np.ascontiguousarray(np.asarray(x, np.float32).astype(BF16))


# ---------------------------------------------------------------------------
# device program
# ---------------------------------------------------------------------------

def _build_program(act_n, variant="full"):
    """variant: 'full' | 'capsule' (skip adapter, copy x->out) |
    'adapter' (skip capsule phases, memset vt) | 'p1'/'p2'/'p3' (capsule
    prefixes: sem only / +squash / +priors)."""
    level = {"p1": 1, "p2": 2, "p3": 3, "capsule": 4, "full": 4, "adapter": 0}[variant]
    nc = bacc.Bacc("TRN2", target_bir_lowering=False, debug=False,
                   num_devices=NCORES)

    d_xtsem = nc.dram_tensor("xt_sem", [6, HK, 128, 3 * S], F32, kind="ExternalInput")
    d_xtown = nc.dram_tensor("xt_own", [HK, 128, TOK], BF, kind="ExternalInput")
    d_xown = nc.dram_tensor("x_own", [BL, S, H], F32, kind="ExternalInput")
    d_rw = nc.dram_tensor("rw_pack", [C, 128, act_n * C * S], F32, kind="ExternalInput")
    d_sw = nc.dram_tensor("sw", [HK, 128, NC30], F32, kind="ExternalInput")
    d_semb = nc.dram_tensor("semb", [1, NC30], F32, kind="ExternalInput")
    d_m1 = nc.dram_tensor("m1", [C, A], BF, kind="ExternalInput")
    d_fc1w = nc.dram_tensor("fc1w", [HK, 128, A], BF, kind="ExternalInput")
    d_fc1b = nc.dram_tensor("fc1b", [128, AK], F32, kind="ExternalInput")
    d_fc2w = nc.dram_tensor("fc2w", [AK, 128, H], BF, kind="ExternalInput")
    d_b2 = nc.dram_tensor("b2row", [1, H], BF, kind="ExternalInput")
    d_g2 = nc.dram_tensor("g2", [1, H], F32, kind="ExternalInput")
    d_masks = nc.dram_tensor("masks", [NPAIR, C], F32, kind="ExternalInput")
    d_vcb = nc.dram_tensor("votecb", [NPAIR * S], BF, kind="Internal")
    d_out = nc.dram_tensor("out", [BL, S, H], F32, kind="ExternalOutput")

    with tile.TileContext(nc) as tc:
        with (
            tc.tile_pool(name="w", bufs=1) as wp,
            tc.tile_pool(name="semx", bufs=2) as sxp,
            tc.tile_pool(name="sem", bufs=1) as smp,
            tc.tile_pool(name="rt", bufs=1) as rp,
            tc.tile_pool(name="ad", bufs=1) as ap_,
            tc.tile_pool(name="st", bufs=2) as sp,
            tc.tile_pool(name="ps", bufs=8, space="PSUM") as pp,
            tc.tile_pool(name="dram", bufs=1, space="DRAM") as dp,
        ):
            # ---------------- persistent weights -----------------
            # (adapter-prepass inputs first: PE can start on fc1 immediately)
            xo_sb = wp.tile([128, HK * TOK], BF, tag="bigx", bufs=1)
            for hk in range(HK):
                nc.scalar.dma_start(xo_sb[:, hk * TOK:(hk + 1) * TOK], d_xtown[hk])
            fc1w_sb = wp.tile([128, HK * A], BF)
            for hk in range(HK):
                nc.scalar.dma_start(fc1w_sb[:, hk * A:(hk + 1) * A], d_fc1w[hk])
            fc1b_sb = wp.tile([128, AK], F32)
            nc.scalar.dma_start(fc1b_sb[:], d_fc1b[:])
            sw_sb = wp.tile([128, HK * NC30], F32)
            for hk in range(HK):
                nc.sync.dma_start(sw_sb[:, hk * NC30:(hk + 1) * NC30], d_sw[hk])
            semb_sb = wp.tile([1, NC30], F32)
            nc.sync.dma_start(semb_sb[:], d_semb[:])
            ones_sb = wp.tile([1, 128], BF)
            nc.gpsimd.memset(ones_sb[:], 1.0)
            ones_f = wp.tile([1, 128], F32)
            nc.gpsimd.memset(ones_f[:], 1.0)
            masks_sb = wp.tile([NPAIR, C], F32)
            nc.sync.dma_start(masks_sb[:], d_masks[:])
            m1_sb = wp.tile([C, A], BF)
            nc.sync.dma_start(m1_sb[:], d_m1[:])
            fc2w_sb = wp.tile([128, AK * H], BF, tag="bigx", bufs=1)
            for ak in range(AK):
                nc.scalar.dma_start(fc2w_sb[:, ak * H:(ak + 1) * H], d_fc2w[ak])
            b2_sb = wp.tile([1, H], BF)
            nc.sync.dma_start(b2_sb[:], d_b2[:])
            g2rep = wp.tile([128, H], F32)
            g2_src = d_g2.ap()  # [1, H] dram -> broadcast to 128 partitions
            g2_b = bass.AP(g2_src.tensor, g2_src.offset, [[0, 128], [1, H]])
            nc.sync.dma_start(g2rep[:], g2_b)

            # ---------------- fc1 pass 1 (x-only part; no routing dep) -----
            # z1p accumulates fc1_w.T @ xT; the capsule term M1.T@VT, bias and
            # relu are applied in pass 2 once routing is done.  Half B is
            # emitted after fc2-A (its z1p slot reuses half A's).
            z1ps = {}

            def emit_fc1_pass1(hf):
                z1p = ap_.tile([128, AK * HALF], BF, tag="z1p", bufs=2,
                               name=f"z1p_{hf}")
                z1ps[hf] = z1p
                for ak in range(AK):
                    pss = [pp.tile([128, 512], F32, tag="mm",
                                   name=f"ps_p1_{hf}_{ak}_{i}") for i in range(2)]
                    for hk in range(HK):
                        lhsT = fc1w_sb[:, hk * A + ak * 128: hk * A + (ak + 1) * 128]
                        for i in range(2):
                            col = hf * HALF + i * 512
                            nc.tensor.matmul(
                                pss[i][:], lhsT,
                                xo_sb[:, hk * TOK + col: hk * TOK + col + 512],
                                start=(hk == 0), stop=(hk == HK - 1))
                    for i in range(2):
                        nc.scalar.copy(
                            z1p[:, ak * HALF + i * 512: ak * HALF + (i + 1) * 512],
                            pss[i][:])


            if variant != "adapter":
                # ------- phase 1: sem + squash (18 examples [16k,16k+18)) -----
                sem_own = smp.tile([128, NSEM * NC30], F32)
                for g in range(6):
                    xt_g = sxp.tile([128, HK * 3 * S], F32, tag="xtg")
                    src_ = d_xtsem.ap()[g]  # [HK, 128, 384]
                    nc.sync.dma_start(
                        xt_g[:].rearrange("p (hk c) -> p hk c", hk=HK),
                        src_.rearrange("hk p c -> p hk c"))
                    for el in range(3):
                        slot = g * 3 + el
                        ps = pp.tile([128, NC30], F32, tag="mm", name=f"ps_sem_{slot}")
                        for hk in range(HK):
                            nc.tensor.matmul(
                                ps[:],
                                xt_g[:, hk * (3 * S) + el * S: hk * (3 * S) + (el + 1) * S],
                                sw_sb[:, hk * NC30:(hk + 1) * NC30],
                                start=(hk == 0), stop=False)
                        nc.tensor.matmul(ps[:], ones_f[:], semb_sb[:],
                                         start=False, stop=True)
                        nc.scalar.copy(sem_own[:, slot * NC30:(slot + 1) * NC30], ps[:])

                # squash over n:  f = sqrt(sq)/(1+sq) via exp(0.5*ln(sq))
                sem2 = smp.tile([128, NSEM * NC30], F32)
                nc.vector.tensor_tensor(sem2[:], sem_own[:], sem_own[:], op=ALU.mult)
                sqt = smp.tile([128, NSEM * C], F32)
                nc.vector.tensor_reduce(
                    sqt[:].rearrange("p (slot cc) -> p slot cc", cc=C),
                    sem2[:].rearrange("p (slot n cc) -> p slot cc n", n=N, cc=C),
                    axis=mybir.AxisListType.X, op=ALU.add)
                lnq = smp.tile([128, NSEM * C], F32)
                nc.scalar.activation(lnq[:], sqt[:], AF.Ln)
                sqq = smp.tile([128, NSEM * C], F32)
                nc.scalar.activation(sqq[:], lnq[:], AF.Exp, scale=0.5)  # sqrt(sq)
                up = smp.tile([128, NSEM * C], F32)
                nc.vector.tensor_scalar_add(up[:], sqt[:], 1.0)
                ru = smp.tile([128, NSEM * C], F32)
                nc.vector.reciprocal(ru[:], up[:])
                fq = smp.tile([128, NSEM * C], F32)
                nc.vector.tensor_tensor(fq[:], sqq[:], ru[:], op=ALU.mult)
                # sem_sq = sem_own * f  (f broadcast over n), fp32
                sem_sq = sem2  # reuse scratch
                f_ap = fq[:]
                f_b = bass.AP(f_ap.tensor, f_ap.offset,
                              [f_ap.ap[0], [C, NSEM], [0, N], [1, C]])
                nc.vector.tensor_tensor(
                    sem_sq[:].rearrange("p (slot n cc) -> p slot n cc", n=N, cc=C),
                    sem_own[:].rearrange("p (slot n cc) -> p slot n cc", n=N, cc=C),
                    f_b, op=ALU.mult)
                # materialize pair-ordered copy: block p=3i+u <- slot i+u
                # (matmul weight APs allow only one free dim, so gather here)
                sem_pair = smp.tile([128, NPAIR * NC30], F32)
                sq_ap = sem_sq[:]
                gather = bass.AP(sq_ap.tensor, sq_ap.offset,
                                 [sq_ap.ap[0], [NC30, BL], [NC30, C], [1, NC30]])
                nc.vector.tensor_copy(
                    sem_pair[:].rearrange("p (i u nc) -> p i u nc", i=BL, u=C),
                    gather)

                if level >= 3:
                    # ---------------- phase 2: priors -----------------
                    # lhsT rows (pair p = 3i+u) read sem slot i+u:
                    # AP dims [(30,16)@i, (30,3)@u] both stride 30 (overlapping)
                    sem_v = sem_pair[:].rearrange("p (pair nc) -> p nc pair", nc=NC30)
                    priors = rp.tile([NPAIR, act_n * S], F32)
                    for g in range(C):
                        for n in range(act_n):
                            ps = pp.tile([NPAIR, S], F32, tag="mm", name=f"ps_pr_{g}_{n}")
                            rwt = sxp.tile([128, C * S], F32, tag="rwt", bufs=4,
                                           name=f"rw_{g}_{n}")
                            nc.scalar.dma_start(
                                rwt[:], d_rw.ap()[g][:, (n * C) * S:(n * C + C) * S])
                            for cc in range(C):
                                nc.tensor.matmul(
                                    ps[:], sem_v[:, n * C + cc, :],
                                    rwt[:, cc * S:(cc + 1) * S],
                                    start=(cc == 0), stop=(cc == C - 1))
                            dst = priors[:, n * S:(n + 1) * S]
                            for g2 in range(1):
                                pass
                            if g == 0:
                                nc.vector.tensor_scalar_mul(dst, ps[:], masks_sb[:, 0:1])
                            else:
                                nc.vector.scalar_tensor_tensor(
                                    dst, ps[:], masks_sb[:, g:g + 1], dst,
                                    op0=ALU.mult, op1=ALU.add)

                if level >= 4:
                    # ---------------- phase 3: routing -----------------
                    vote = rp.tile([NPAIR, S], F32)
                    scr = rp.tile([NPAIR, S], F32)
                    La = rp.tile([NPAIR, act_n], F32)
                    Lb = rp.tile([NPAIR, act_n], F32)
                    sqv = rp.tile([NPAIR, 1], F32)
                    lv = rp.tile([NPAIR, 1], F32)
                    sv = rp.tile([NPAIR, 1], F32)
                    uv = rp.tile([NPAIR, 1], F32)
                    rv = rp.tile([NPAIR, 1], F32)
                    fv = rp.tile([NPAIR, 1], F32)
                    outv = rp.tile([NPAIR, S], F32)
                    mx = rp.tile([NPAIR, 1], F32)
                    mneg = rp.tile([NPAIR, 1], F32)
                    ex = rp.tile([NPAIR, act_n], F32)
                    es = rp.tile([NPAIR, 1], F32)
                    ers = rp.tile([NPAIR, 1], F32)
                    probs = rp.tile([NPAIR, act_n], F32)

                    def vote_from(pr_scalar_ap_or_const, first_const=None):
                        """vote = sum_n probs_n * priors_n."""
                        for n in range(act_n):
                            blk = priors[:, n * S:(n + 1) * S]
                            sc = (first_const if first_const is not None
                                  else pr_scalar_ap_or_const[:, n:n + 1])
                            if n == 0:
                                nc.vector.tensor_scalar_mul(vote[:], blk, sc)
                            else:
                                nc.vector.scalar_tensor_tensor(
                                    vote[:], blk, sc, vote[:], op0=ALU.mult, op1=ALU.add)

                    def squash_vote():
                        nc.vector.tensor_tensor(scr[:], vote[:], vote[:], op=ALU.mult)
                        nc.vector.tensor_reduce(sqv[:], scr[:],
                                                axis=mybir.AxisListType.X, op=ALU.add)
                        nc.scalar.activation(lv[:], sqv[:], AF.Ln)
                        nc.scalar.activation(sv[:], lv[:], AF.Exp, scale=0.5)
                        nc.vector.tensor_scalar_add(uv[:], sqv[:], 1.0)
                        nc.vector.reciprocal(rv[:], uv[:])
                        nc.vector.tensor_tensor(fv[:], sv[:], rv[:], op=ALU.mult)
                        nc.vector.tensor_scalar_mul(outv[:], vote[:], fv[:])

                    def deltas(Lprev, Lnew, first):
                        for n in range(act_n):
                            nc.vector.tensor_tensor(
                                scr[:], priors[:, n * S:(n + 1) * S], outv[:],
                                op=ALU.mult)
                            nc.vector.tensor_reduce(
                                Lnew[:, n:n + 1], scr[:],
                                axis=mybir.AxisListType.X, op=ALU.add)
                        if not first:
                            nc.vector.tensor_tensor(Lnew[:], Lnew[:], Lprev[:],
                                                    op=ALU.add)

                    def softmax(L):
                        nc.vector.tensor_reduce(mx[:], L[:], axis=mybir.AxisListType.X,
                                                op=ALU.max)
                        nc.vector.tensor_scalar_mul(mneg[:], mx[:], -1.0)
                        nc.scalar.activation(ex[:], L[:], AF.Exp, bias=mneg[:],
                                             accum_out=es[:])
                        nc.vector.reciprocal(ers[:], es[:])
                        nc.vector.tensor_scalar_mul(probs[:], ex[:], ers[:])

                    # iter 0
                    vote_from(None, first_const=1.0 / act_n)
                    squash_vote()
                    deltas(None, La, first=True)
                    # iter 1
                    softmax(La)
                    vote_from(probs)
                    squash_vote()
                    deltas(La, Lb, first=False)
                    # iter 2 (final)
                    softmax(Lb)
                    vote_from(probs)

                    vb = rp.tile([NPAIR, S], BF)
                    nc.vector.tensor_copy(vb[:], vote[:])
                    nc.sync.dma_start(
                        d_vcb.ap().rearrange("(p s) -> p s", p=NPAIR), vb[:])

                    # VT[c, e*128+s] = votecb_flat[3*e*128 + 3*s + c]
                    vt_sb = ap_.tile([C, TOK], BF)
                    vflat = d_vcb.ap()
                    for e in range(BL):
                        src = bass.AP(vflat.tensor, vflat.offset + 3 * e * S,
                                      [[1, C], [C, S]])
                        nc.sync.dma_start(vt_sb[:, e * S:(e + 1) * S], src)

            else:
                vt_sb = ap_.tile([C, TOK], BF)
                nc.gpsimd.memset(vt_sb[:], 0.0)

            if variant in ("full", "adapter"):
                emit_fc1_pass1(0)
                emit_fc1_pass1(1)
                # -------- phase 4: fc1 pass 2 (capsule term) + fc2 --------
                def emit_fc1_pass2_and_fc2(hf):
                    z1 = z1ps[hf]
                    for ak in range(AK):
                        ps2 = [pp.tile([128, 512], F32, tag="mm",
                                       name=f"ps_p2_{hf}_{ak}_{i}") for i in range(2)]
                        m1l = m1_sb[:, ak * 128:(ak + 1) * 128]
                        for i in range(2):
                            col = hf * HALF + i * 512
                            nc.tensor.matmul(ps2[i][:], m1l,
                                             vt_sb[:, col:col + 512],
                                             start=True, stop=True)
                        tmp = sp.tile([128, HALF], F32, tag="tmp",
                                      name=f"tmp_{hf}_{ak}", bufs=2)
                        for i in range(2):
                            # tmp = (ps2 + fc1b) + z1p
                            nc.vector.scalar_tensor_tensor(
                                tmp[:, i * 512:(i + 1) * 512], ps2[i][:],
                                fc1b_sb[:, ak:ak + 1],
                                z1[:, ak * HALF + i * 512: ak * HALF + (i + 1) * 512],
                                op0=ALU.add, op1=ALU.add)
                        nc.scalar.activation(
                            z1[:, ak * HALF:(ak + 1) * HALF], tmp[:], AF.Relu)
                    for tt in range(8):
                        e = hf * 8 + tt
                        psa = pp.tile([128, 512], F32, tag="mm", name=f"ps_f2a_{e}")
                        psb = pp.tile([128, 256], F32, tag="mm", name=f"ps_f2b_{e}")
                        for ak in range(AK):
                            lhsT = z1[:, ak * HALF + tt * 128: ak * HALF + (tt + 1) * 128]
                            nc.tensor.matmul(psa[:], lhsT,
                                             fc2w_sb[:, ak * H: ak * H + 512],
                                             start=(ak == 0), stop=False)
                            nc.tensor.matmul(psb[:], lhsT,
                                             fc2w_sb[:, ak * H + 512: ak * H + H],
                                             start=(ak == 0), stop=False)
                        nc.tensor.matmul(psa[:], ones_sb[:], b2_sb[:, 0:512],
                                         start=False, stop=True)
                        nc.tensor.matmul(psb[:], ones_sb[:], b2_sb[:, 512:H],
                                         start=False, stop=True)
                        xt = sp.tile([128, H], F32, tag="x", name=f"x_{e}")
                        nc.sync.dma_start(xt[:], d_xown[e])
                        ot = sp.tile([128, H], F32, tag="o", name=f"o_{e}")
                        nc.scalar.activation(ot[:, 0:512], psa[:], AF.Relu)
                        nc.scalar.activation(ot[:, 512:H], psb[:], AF.Relu)
                        nc.vector.tensor_tensor(ot[:], ot[:], g2rep[:], op=ALU.mult)
                        nc.vector.tensor_tensor(ot[:], ot[:], xt[:], op=ALU.add)
                        nc.sync.dma_start(d_out[e], ot[:])

                emit_fc1_pass2_and_fc2(0)
                emit_fc1_pass2_and_fc2(1)
            else:
                for e in range(BL):
                    xt = sp.tile([128, H], F32, tag="x", name=f"xc_{e}")
                    nc.sync.dma_start(xt[:], d_xown[e])
                    nc.sync.dma_start(d_out[e], xt[:])

    nc.compile()
    return nc


# ---------------------------------------------------------------------------
# host marshaling
# ---------------------------------------------------------------------------

def _prep_core_inputs(k, x, shared, act_n):
    # own (output) examples: b_i = 48k + 43 i (mod 128).  Because
    # 3*43 = 129 = 1 (mod 128), the 48 routing pairs t = 3 b_i + u map to
    # sem examples b' = t mod 128 = 16k + (i + u) mod 128 — just the 18
    # consecutive examples [16k, 16k+18).  Pair (i,u) sits at row 3i+u and
    # reads sem slot i+u; votecb rows 3e..3e+2 are exactly what h_out of
    # own example e needs, so no cross-core traffic anywhere.
    own = np.array([(48 * k + 43 * i) % B for i in range(BL)])
    sem_ex = np.array([(16 * k + j) % B for j in range(NSEM)])

    # xt_sem: [6, hk, 128, 3*S] fp32, groups of 3 sem examples
    xs = np.transpose(x[sem_ex], (2, 0, 1)).reshape(H, NSEM * S).astype(np.float32)
    xt_sem = np.empty((6, HK, 128, 3 * S), np.float32)
    for g in range(6):
        for hk in range(HK):
            xt_sem[g, hk] = xs[hk * 128:(hk + 1) * 128,
                               g * 3 * S:(g + 1) * 3 * S]

    xo = np.transpose(x[own], (2, 0, 1)).reshape(H, TOK).astype(BF16)
    xt_own = np.ascontiguousarray(xo.reshape(HK, 128, TOK))
    x_own = np.ascontiguousarray(x[own].astype(np.float32))

    # group g == c' directly; mask[p, g] = (c' of pair p == g)
    masks = np.zeros((NPAIR, C), np.float32)
    for i in range(BL):
        for u in range(C):
            t = 3 * int(own[i]) + u
            masks[3 * i + u, t // B] = 1.0

    return {
        "xt_sem": xt_sem,
        "xt_own": xt_own,
        "x_own": x_own,
        "rw_pack": shared["rw_pack"],
        "masks": masks,
        **{n: shared[n] for n in ("sw", "semb", "m1", "fc1w", "fc1b",
                                  "fc2w", "b2row", "g2")},
    }


_CACHE = {}


def _make_shared(inputs):
    fc1_w = np.asarray(inputs["fc1_w"], np.float32)
    fc1_b = np.asarray(inputs["fc1_b"], np.float32)
    fc2_w = np.asarray(inputs["fc2_w"], np.float32)
    fc2_b = np.asarray(inputs["fc2_b"], np.float32)
    efc1 = np.asarray(inputs["efc1"], np.float32)
    efc2 = np.asarray(inputs["efc2"], np.float32)
    sem_w = np.asarray(inputs["sem_w"], np.float32)
    sem_b = np.asarray(inputs["sem_b"], np.float32)
    route_weights = np.asarray(inputs["route_weights"], np.float32)
    larger_w = np.asarray(inputs["larger_w"], np.float32)
    larger_b = np.asarray(inputs["larger_b"], np.float32)
    elarger = np.asarray(inputs["elarger"], np.float32)
    t = int(np.asarray(inputs["t"]))
    sf = np.float32(int(np.asarray(inputs["s"])))
    act_n = t + 1

    gfc1 = _sigmoid_f32(sf * efc1[t])
    gfc2 = _sigmoid_f32(sf * efc2[t])
    glarger = _sigmoid_f32(sf * elarger[t])

    lwg = (larger_w * glarger[None, :]).astype(np.float32)
    lb_eff = (larger_b * glarger).astype(np.float32)
    rw4 = route_weights.reshape(C, N, S, C, S)

    return {
        "sw": np.ascontiguousarray(np.transpose(sem_w, (1, 0, 2))
                                   .reshape(H, NC30).astype(np.float32)
                                   ).reshape(HK, 128, NC30),
        "semb": np.ascontiguousarray(sem_b.reshape(1, NC30).astype(np.float32)),
        "m1": _bf(lwg @ fc1_w),
        "fc1w": _bf(fc1_w).reshape(HK, 128, A),
        "fc1b": np.ascontiguousarray(
            (fc1_b + lb_eff @ fc1_w).astype(np.float32).reshape(AK, 128).T),
        "fc2w": _bf(fc2_w * gfc1[:, None]).reshape(AK, 128, H),
        "b2row": _bf(fc2_b.reshape(1, H)),
        "g2": np.ascontiguousarray(gfc2.reshape(1, H)),
        "rw_pack": np.stack([
            np.ascontiguousarray(np.transpose(rw4[c, :act_n], (1, 0, 2, 3))
                                 .reshape(S, act_n * C * S).astype(np.float32))
            for c in range(C)]),
    }


def kernel(**inputs):
    x = np.asarray(inputs["x"], np.float32)
    t = int(np.asarray(inputs["t"]))
    act_n = t + 1
    shared = _make_shared(inputs)

    if act_n not in _CACHE:
        _CACHE[act_n] = _build_program(act_n)
    nc = _CACHE[act_n]

    in_maps = [_prep_core_inputs(k, x, shared, act_n) for k in range(NCORES)]
    res = bass_utils.run_bass_kernel_spmd(nc, in_maps, core_ids=list(range(NCORES)))
    out = np.empty((B, S, H), np.float32)
    for k in range(NCORES):
        own = [(48 * k + 43 * i) % B for i in range(BL)]
        out[own] = res.results[k]["out"]
    return out

